# revision 23
# baseline (speedup 1.0000x reference)
"""Trainium2 Bass kernel for nn_MultiHeadAttention_68865505624655.

Strategy (head parallelism, 8 cores x 2 heads, bf16 pipeline):
  The reference's reshape(B,-1,T,H) mixes time/channel dims. For head h the
  per-head matrices are exactly reinterpretations of the compacted projection
  output Y_h = X @ W[h::16].T (shape (3072, 64)):
      Q_h^T (xi, t2)  == Y_h viewed as (64, 3072)   (same linear memory!)
      K_h^T (xi, t2)  == same
      V_h  (t2', xi)  == transpose of that view     (needs a real transpose)
  log2e/128 is folded into Wq host-side, so the "energy" the PE produces is
  E' = E * log2e/128 -- the form both exp paths below want.
  Each core:
    1. fused QKV projection for its 2 heads in bf16: Y6 = X @ W6^T + b
       (24 t-blocks x 8 k-tiles), written to DRAM scratch (bf16 planes).
    2. reads back Q^T/K^T/V^T as contiguous (64,3072) views (one 6KB
       descriptor per partition); V tiles via PE transposes (4 per DVE copy).
    3. per c-tile, the two heads' energy matmuls (K=64) go back-to-back into
       one PSUM tile at tile_position (0,0)/(64,0) -> the PE runs the pair
       concurrently in disjoint row groups (~1.75x).  exp(E) alternates
       between ScalarE (Exp activation, scale=128*ln2) and the Vector engine
       (Schraudolph bit-trick: ONE tensor_scalar computing
       int16(round(E'*2^14 + B)) whose bits are the bf16 approximation of
       exp(E), ~3% max rel err) so neither engine is the softmax bottleneck.
       One bf16 matmul per (c,head) with lhsT = [V_c | 1] (M=65) accumulates
       BOTH out^T[xi,r] and the softmax denominator Sigma[r] (row 64).
    4. writes per-head [out^T; Sigma] (65,3072) tiles per core.
  Host: divide rows 0:64 by row 64, interleave heads into (T,D), gamma*out+x.
  Toolchain workarounds: _split_multiwaits (this walrus allows one sync wait
  per instruction) and _install_ntff_shim (axon NTFF profiling hook).
"""

import sys

if "/opt/trn_rl_repo" not in sys.path:
    sys.path.insert(0, "/opt/trn_rl_repo")

import ml_dtypes
import numpy as np

BF16NP = ml_dtypes.bfloat16


def _install_ntff_shim():
    """concourse.bass_utils under axon imports antenv.axon_hooks when
    tracing is requested; this image's antenv lacks that submodule.
    Register an equivalent shim (backed by the boot image's ctypes NTFF
    driver) so BASS_TRACE=1 profiles instead of crashing."""
    import types

    if "antenv.axon_hooks" in sys.modules:
        return
    mod = types.ModuleType("antenv.axon_hooks")
    cell = {}

    def get_axon_ntff_profile_hook():
        if "h" not in cell:
            try:
                from trn_agent_boot.trn_boot import _ntff_profile_via_ctypes
                cell["h"] = _ntff_profile_via_ctypes("/opt/axon/libaxon_pjrt.so")
            except Exception:
                cell["h"] = None
        return cell["h"]

    def set_axon_ntff_profile_hook(h):
        cell["h"] = h

    mod.get_axon_ntff_profile_hook = get_axon_ntff_profile_hook
    mod.set_axon_ntff_profile_hook = set_axon_ntff_profile_hook
    sys.modules["antenv.axon_hooks"] = mod


_install_ntff_shim()

import concourse.bass as bass
import concourse.mybir as mybir
import concourse.tile as tile
from concourse.bass import ds, ts
from concourse.masks import make_identity

QSCALE = 1.4426950408889634 / 128.0   # log2e / 128 folded into Wq
ACT_EXP_SCALE = 128.0 * 0.6931471805599453  # recovers exp(E) on ScalarE

F32 = mybir.dt.float32
F32R = mybir.dt.float32r
BF16 = mybir.dt.bfloat16

T = 3072          # sequence length (and t2 size)
D = 1024          # model dim
H = 16            # heads
NCORE = 8
EG = 64           # channel groups per head (columns of Y_h)
XI = 64           # "feature" dim of the quirky attention (t // 48)
NKT = D // 128    # 8 contraction tiles for the projection
NTB = T // 128    # 24 t-blocks / c-tiles
RCH = 512         # r-chunk (free dim of energy/AV matmuls)
NR = T // RCH     # 6 r-chunks
W6 = 6 * EG       # 384 fused projection output columns
# c-tiles whose softmax exp runs on the Vector engine (Schraudolph bit-trick
# via one tensor_scalar) instead of ScalarE -- splits the softmax exp load.
DVE_SET = frozenset({1, 3, 5, 7, 9, 11, 13, 15, 17, 19, 21})
SCHRAUD_A = float(2.0 ** 14)          # 2^7 * 128 (E' is prescaled by 1/128)
SCHRAUD_B = 16250.368                 # (127 - 0.044) * 2^7


def _split_multiwaits(nc):
    """This toolchain's walrus accepts at most ONE sync wait per
    instruction (setupSyncWait: 'Too many sync wait commands'), but Tile
    attaches several. Hoist all but the last wait of each instruction onto
    same-engine NoOps inserted right before it — semantically identical
    (sem-ge waits executed in sequence)."""
    n = 0
    for fn in nc.m.functions:
        for bb in fn.blocks:
            insts = list(bb.instructions)
            out = []
            changed = False
            for inst in insts:
                si = inst.sync_info
                if si is not None and len(si.on_wait) > 1:
                    waits = list(si.on_wait)
                    for w in waits[:-1]:
                        n += 1
                        out.append(mybir.InstNoOp(
                            name=f"I-splitwait-{n}",
                            ins=[], outs=[], engine=inst.engine,
                            sync_info=mybir.SyncInfo(on_wait=[w], on_update=[]),
                        ))
                    inst.sync_info = mybir.SyncInfo(
                        on_wait=[waits[-1]], on_update=list(si.on_update)
                    )
                    changed = True
                out.append(inst)
            if changed:
                bb.instructions = out
    return n


def _coarsen_sem_incs(nc):
    """Tile attaches a sem-inc to EVERY instruction (its optimize_sems pass
    is disabled).  Each inc is a serializing EVT_SEM register write (~26ns)
    and breaks PE row-group matmul pairing.  Drop incs whose cumulative
    values no wait ever references, folding the dropped counts into the next
    kept inc -- sem values at every waited point are unchanged.  DMA sems
    (completion-fired) are left untouched."""
    import collections

    waitvals = collections.defaultdict(set)
    badsem = set()
    for fn in nc.m.functions:
        for bb in fn.blocks:
            for inst in bb.instructions:
                si = inst.sync_info
                if si is None:
                    continue
                for w in si.on_wait:
                    if (
                        w.sync_type != "semaphore"
                        or w.wait_mode != "sem-ge-imm"
                        or w.wait_reg is not None
                    ):
                        badsem.add(w.id)
                    else:
                        waitvals[w.id].add(w.wait_value)

    upd_sites = collections.defaultdict(list)
    for fn in nc.m.functions:
        for bb in fn.blocks:
            for inst in bb.instructions:
                si = inst.sync_info
                if si is None:
                    continue
                for u in si.on_update:
                    if (
                        u.sync_type != "semaphore"
                        or u.update_mode != "sem-inc"
                        or u.update_reg is not None
                        or (u.ant_name or "").startswith("DMA")
                    ):
                        badsem.add(u.id)
                        continue
                    upd_sites[u.id].append((id(bb), inst, u))

    for sid, sites in upd_sites.items():
        if len({b for b, _, _ in sites}) > 1:
            badsem.add(sid)
        if len({inst.engine for _, inst, _ in sites}) > 1:
            badsem.add(sid)

    # walrus requires every sem-inc to have update_value == 1, so dropped
    # incs cannot be folded into a bigger one.  Instead drop them outright
    # and renumber every wait on that sem to the new (smaller) counts.
    ndrop = 0
    dropset = set()          # ids of SyncUpdate objects to drop
    remap = {}               # sem id -> sorted list of kept cum values
    for sid, sites in upd_sites.items():
        if sid in badsem:
            continue
        wv = waitvals.get(sid, set())
        cum = 0
        kept_cums = []
        for i, (_, inst, u) in enumerate(sites):
            cum += u.update_value
            if (cum in wv) or (i == len(sites) - 1):
                kept_cums.append(cum)
            else:
                dropset.add(id(u))
                ndrop += 1
        remap[sid] = kept_cums

    import bisect

    for fn in nc.m.functions:
        for bb in fn.blocks:
            for inst in bb.instructions:
                si = inst.sync_info
                if si is None:
                    continue
                new_upd = [u for u in si.on_update if id(u) not in dropset]
                new_wait = []
                for w in si.on_wait:
                    if w.id in remap:
                        kept = remap[w.id]
                        # new value = rank of first kept cum >= old value
                        nv = bisect.bisect_left(kept, w.wait_value) + 1
                        nv = min(nv, len(kept))
                        if nv != w.wait_value:
                            w = mybir.SyncWait(
                                sync_type=w.sync_type,
                                id=w.id,
                                ant_name=w.ant_name,
                                wait_mode=w.wait_mode,
                                wait_value=nv,
                                wait_reg=w.wait_reg,
                            )
                    new_wait.append(w)
                if len(new_upd) != len(si.on_update) or any(
                    a is not b for a, b in zip(new_wait, si.on_wait)
                ):
                    inst.sync_info = mybir.SyncInfo(
                        on_wait=new_wait, on_update=new_upd
                    )
    return ndrop


def build_program():
    nc = bass.Bass()

    xT = nc.dram_tensor("xT", [NTB, 128, NKT, 128], BF16, kind="ExternalInput")
    w6 = nc.dram_tensor("w6", [D, W6], BF16, kind="ExternalInput")
    b6 = nc.dram_tensor("b6", [128, W6], F32, kind="ExternalInput")
    y6qk = nc.dram_tensor("y6qk", [4, T, EG], BF16, kind="Internal")
    y6v = nc.dram_tensor("y6v", [2, T, EG], BF16, kind="Internal")
    outT = nc.dram_tensor("outT", [2, XI + 1, T], F32, kind="ExternalOutput")

    with tile.TileContext(nc) as tc:
        with tc.tile_pool(name="const", bufs=1) as constp:
            w6_sb = constp.tile([128, NKT, W6], BF16)
            w6v = w6[:, :].rearrange("(k p) n -> k p n", p=128)
            for k in range(NKT):
                nc.scalar.dma_start(out=w6_sb[:, k, :], in_=w6v[k, :, :])
            b6_sb = constp.tile([128, W6], F32)
            nc.scalar.dma_start(out=b6_sb, in_=b6[:, :])
            # identity blocks at partitions 0:64 and 64:128 so the two heads'
            # V^T transposes run row-paired in the PE array
            ident = constp.tile([128, 64], BF16)
            nc.gpsimd.memset(ident, 0.0)
            make_identity(nc, ident[0:64, :], nomemset=True)
            make_identity(nc, ident[64:128, :], nomemset=True)
            ones_f32 = constp.tile([128, 1], F32)
            nc.gpsimd.memset(ones_f32, 1.0)
            kt_sb = constp.tile([128, T], BF16)   # rows 0:64 h1 K^T, 64:128 h2
            qt_sb = constp.tile([128, T], BF16)   # rows 0:64 h1 Q^T, 64:128 h2
            vt_sb = constp.tile([128, T], BF16)   # rows 0:64 h1 V^T, 64:128 h2
            # V tiles augmented with a ones column: [:, c, 0:64] = V_h c-tile,
            # [:, c, 64] = 1.0 so one matmul computes out^T AND Sigma (row 64)
            v1_sb = constp.tile([128, NTB, XI + 1], BF16)
            v2_sb = constp.tile([128, NTB, XI + 1], BF16)
            for vsb in (v1_sb, v2_sb):
                for c in range(NTB):
                    nc.vector.tensor_copy(vsb[:, c, XI:XI + 1], ones_f32)

            # ---------------- projection: Y6 = X @ W6^T + b6 ----------------
            with tc.tile_pool(name="xt", bufs=6) as xtp, \
                 tc.tile_pool(name="psy", bufs=4, space="PSUM") as psyp, \
                 tc.tile_pool(name="ysb", bufs=4) as ysbp:
                for j in range(NTB):
                    xt = xtp.tile([128, NKT, 128], BF16)
                    (nc.sync if j % 2 == 0 else nc.scalar).dma_start(
                        out=xt.rearrange("p k t -> p (k t)"),
                        in_=xT[j, :, :, :].rearrange("p k t -> p (k t)"),
                    )
                    psy = psyp.tile([128, W6], F32)
                    for k in range(NKT):
                        nc.tensor.matmul(
                            psy, xt[:, k, :], w6_sb[:, k, :],
                            start=(k == 0), stop=(k == NKT - 1),
                        )
                    psyv = psy.rearrange("p (h z e) -> p h z e", h=2, z=3)
                    b6v = b6_sb.rearrange("p (h z e) -> p h z e", h=2, z=3)
                    ysbqk = ysbp.tile([128, 2, 2, EG], BF16, name="ysbqk")
                    nc.vector.tensor_add(ysbqk, psyv[:, :, 0:2, :],
                                         b6v[:, :, 0:2, :])
                    ysbv = ysbp.tile([128, 2, EG], BF16, name="ysbv")
                    nc.vector.tensor_add(ysbv, psyv[:, :, 2, :],
                                         b6v[:, :, 2, :])
                    nc.scalar.dma_start(
                        out=y6qk[:, ts(j, 128), :].rearrange("q t e -> t q e"),
                        in_=ysbqk,
                    )
                    nc.sync.dma_start(
                        out=y6v[:, ts(j, 128), :].rearrange("q t e -> t q e"),
                        in_=ysbv,
                    )

            # ------- load Q^T / K^T / V^T as contiguous (64, 3072) views.
            # Flat 2D APs -> one 6KB descriptor per partition (NOT 48x128B).
            engs = (nc.sync, nc.scalar)
            ldn = 0
            for srcap, bufap in (
                    (y6v[0, :, :], vt_sb[0:64, :]),
                    (y6v[1, :, :], vt_sb[64:128, :]),
                    (y6qk[1, :, :], kt_sb[0:64, :]),
                    (y6qk[3, :, :], kt_sb[64:128, :]),
                    (y6qk[0, :, :], qt_sb[0:64, :]),
                    (y6qk[2, :, :], qt_sb[64:128, :])):
                engs[ldn % 2].dma_start(
                    out=bufap,
                    in_=srcap.rearrange("(xi a) e -> xi (a e)", xi=64),
                )
                ldn += 1

            # ------- V tiles: true transpose of V^T chunks via the PE -------
            # 4 c-tiles batched per PSUM tile so one DVE copy moves 4 blocks
            with tc.tile_pool(name="vtps", bufs=4, space="PSUM") as vtpsp:
                for c0 in range(0, NTB, 4):
                    for vsb, row0 in ((v1_sb, 0), (v2_sb, 64)):
                        vp = vtpsp.tile([128, 4, XI], BF16)
                        for i in range(4):
                            nc.tensor.transpose(
                                vp[:, i, :],
                                vt_sb[row0:row0 + 64, ts(c0 + i, 128)],
                                ident[row0:row0 + 64, :],
                            )
                        nc.vector.tensor_copy(
                            vsb[:, ds(c0, 4), 0:XI], vp
                        )

            # --------------------------- attention --------------------------
            # Per c-tile the two heads' energy matmuls are emitted back to
            # back into the same PSUM tile (disjoint PE row groups 0:64 /
            # 64:128 -> the array runs them concurrently).  Each (128, 1024)
            # energy pair is exponentiated on ScalarE (exp) OR on the Vector
            # engine (custom poly^128 ops) per DVE_SET to split the softmax
            # exp across both engines.
            with tc.tile_pool(name="eps", bufs=3, space="PSUM") as epp, \
                 tc.tile_pool(name="ex", bufs=6) as expool, \
                 tc.tile_pool(name="outp", bufs=1, space="PSUM") as outpp, \
                 tc.tile_pool(name="osb", bufs=4) as osbp:
                for r in range(NR):
                    qt = qt_sb[:, ts(r, RCH)]
                    outp1 = outpp.tile([XI + 1, RCH], F32)
                    outp2 = outpp.tile([XI + 1, RCH], F32)
                    for c in range(NTB):
                        ep = epp.tile([128, 2, RCH], F32)
                        nc.tensor.matmul(
                            ep[:, 0, :], kt_sb[0:64, ts(c, 128)],
                            qt[0:64, :], start=True, stop=True,
                        )
                        nc.tensor.matmul(
                            ep[:, 1, :], kt_sb[64:128, ts(c, 128)],
                            qt[64:128, :], start=True, stop=True,
                        )
                        ex = expool.tile([128, 2, RCH], BF16)
                        if c in DVE_SET:
                            # Schraudolph exp on the DVE: one tensor_scalar
                            # computes i16(round(E'*2^14 + B)); the int bits
                            # ARE the bf16 approximation of exp(E).
                            nc.vector.tensor_scalar(
                                ex.bitcast(mybir.dt.int16), ep,
                                SCHRAUD_A, SCHRAUD_B,
                                mybir.AluOpType.mult, mybir.AluOpType.add,
                            )
                        else:
                            nc.scalar.activation(
                                ex, ep, mybir.ActivationFunctionType.Exp,
                                scale=ACT_EXP_SCALE,
                            )
                        nc.tensor.matmul(
                            outp1, v1_sb[:, c, :], ex[:, 0, :],
                            start=(c == 0), stop=(c == NTB - 1),
                        )
                        nc.tensor.matmul(
                            outp2, v2_sb[:, c, :], ex[:, 1, :],
                            start=(c == 0), stop=(c == NTB - 1),
                        )
                    for outp, hl in ((outp1, 0), (outp2, 1)):
                        osb = osbp.tile([XI + 1, RCH], F32)
                        if hl == 0:
                            nc.vector.tensor_copy(osb, outp)
                        else:
                            nc.scalar.copy(osb, outp)
                        nc.gpsimd.dma_start(
                            out=outT[hl, :, ts(r, RCH)], in_=osb
                        )
    return nc


def make_in_maps(x, Wq, bq, Wk, bk, Wv, bv):
    X = np.ascontiguousarray(np.asarray(x, dtype=np.float32).reshape(T, D))
    # (NTB, 128, NKT, 128): [j, p, k, t] = X[128j+t, 128k+p] -- every SBUF
    # partition reads one contiguous 2KB run per projection slab DMA
    xTm = np.ascontiguousarray(
        X.astype(BF16NP).reshape(NTB, 128, NKT, 128).transpose(0, 3, 2, 1)
    )
    in_maps = []
    for c in range(NCORE):
        wcols, bcols = [], []
        for h in (2 * c, 2 * c + 1):
            for W, b, s in ((Wq, bq, QSCALE), (Wk, bk, 1.0), (Wv, bv, 1.0)):
                wcols.append(np.asarray(W, np.float32)[h::H, :].T * np.float32(s))
                bcols.append(np.asarray(b, np.float32)[h::H] * np.float32(s))
        w6m = np.ascontiguousarray(np.concatenate(wcols, axis=1).astype(BF16NP))
        b6m = np.ascontiguousarray(
            np.broadcast_to(np.concatenate(bcols), (128, W6)).astype(np.float32)
        )
        in_maps.append({"xT": xTm, "w6": w6m, "b6": b6m})
    return X, in_maps


def assemble(X, results, gamma):
    O = np.empty((T, EG, H), dtype=np.float32)
    for c in range(NCORE):
        res = results[c]
        for hl in range(2):
            h = 2 * c + hl
            onn = res["outT"][hl][0:XI, :]                # (64, 3072)
            s = res["outT"][hl][XI, :]                    # (3072,)
            O[:, :, h] = (onn / s[None, :]).T
    out = O.reshape(T, D)
    g = np.float32(np.asarray(gamma))
    return (g * out + X).reshape(1, 1, T, D).astype(np.float32)


_PROGRAM = None
last_run_info = {}


def kernel(x, Wq, bq, Wk, bk, Wv, bv, gamma):
    global _PROGRAM
    from concourse import bass_utils

    X, in_maps = make_in_maps(x, Wq, bq, Wk, bk, Wv, bv)
    if _PROGRAM is None:
        import os

        _PROGRAM = build_program()
        if os.environ.get("COARSEN") == "1":
            _coarsen_sem_incs(_PROGRAM)
        # required for this toolchain's walrus (1 sync wait per instruction);
        # applied here so CoreSim (which predates these NoOps) can still run
        # the unsplit program from build_program()
        _split_multiwaits(_PROGRAM)
    res = bass_utils.run_bass_kernel_spmd(
        _PROGRAM, in_maps, core_ids=list(range(NCORE))
    )
    last_run_info["exec_time_ns"] = res.exec_time_ns
    last_run_info["trace"] = res.instructions_and_trace
    return assemble(X, res.results, gamma)



# revision 26
# speedup vs baseline: 1.0091x; 1.0091x over previous
"""Trainium2 Bass kernel for nn_MultiHeadAttention_68865505624655.

Strategy (head parallelism, 8 cores x 2 heads, bf16 pipeline):
  The reference's reshape(B,-1,T,H) mixes time/channel dims. For head h the
  per-head matrices are exactly reinterpretations of the compacted projection
  output Y_h = X @ W[h::16].T (shape (3072, 64)):
      Q_h^T (xi, t2)  == Y_h viewed as (64, 3072)   (same linear memory!)
      K_h^T (xi, t2)  == same
      V_h  (t2', xi)  == transpose of that view     (needs a real transpose)
  log2e/128 is folded into Wq host-side, so the "energy" the PE produces is
  E' = E * log2e/128 -- the form both exp paths below want.
  Each core:
    1. fused QKV projection for its 2 heads in bf16: Y6 = X @ W6^T + b
       (24 t-blocks x 8 k-tiles), written to DRAM scratch (bf16 planes).
    2. reads back Q^T/K^T/V^T as contiguous (64,3072) views (one 6KB
       descriptor per partition); V tiles via PE transposes (4 per DVE copy).
    3. per c-tile, the two heads' energy matmuls (K=64) go back-to-back into
       one PSUM tile at tile_position (0,0)/(64,0) -> the PE runs the pair
       concurrently in disjoint row groups (~1.75x).  exp(E) alternates
       between ScalarE (Exp activation, scale=128*ln2) and the Vector engine
       (Schraudolph bit-trick: ONE tensor_scalar computing
       int16(round(E'*2^14 + B)) whose bits are the bf16 approximation of
       exp(E), ~3% max rel err) so neither engine is the softmax bottleneck.
       One bf16 matmul per (c,head) with lhsT = [V_c | 1] (M=65) accumulates
       BOTH out^T[xi,r] and the softmax denominator Sigma[r] (row 64).
    4. writes per-head [out^T; Sigma] (65,3072) tiles per core.
  Host: divide rows 0:64 by row 64, interleave heads into (T,D), gamma*out+x.
  Toolchain workarounds: _split_multiwaits (this walrus allows one sync wait
  per instruction) and _install_ntff_shim (axon NTFF profiling hook).
"""

import sys

if "/opt/trn_rl_repo" not in sys.path:
    sys.path.insert(0, "/opt/trn_rl_repo")

import ml_dtypes
import numpy as np

BF16NP = ml_dtypes.bfloat16


def _install_ntff_shim():
    """concourse.bass_utils under axon imports antenv.axon_hooks when
    tracing is requested; this image's antenv lacks that submodule.
    Register an equivalent shim (backed by the boot image's ctypes NTFF
    driver) so BASS_TRACE=1 profiles instead of crashing."""
    import types

    if "antenv.axon_hooks" in sys.modules:
        return
    mod = types.ModuleType("antenv.axon_hooks")
    cell = {}

    def get_axon_ntff_profile_hook():
        if "h" not in cell:
            try:
                from trn_agent_boot.trn_boot import _ntff_profile_via_ctypes
                cell["h"] = _ntff_profile_via_ctypes("/opt/axon/libaxon_pjrt.so")
            except Exception:
                cell["h"] = None
        return cell["h"]

    def set_axon_ntff_profile_hook(h):
        cell["h"] = h

    mod.get_axon_ntff_profile_hook = get_axon_ntff_profile_hook
    mod.set_axon_ntff_profile_hook = set_axon_ntff_profile_hook
    sys.modules["antenv.axon_hooks"] = mod


_install_ntff_shim()

import concourse.bass as bass
import concourse.mybir as mybir
import concourse.tile as tile
from concourse.bass import ds, ts
from concourse.masks import make_identity

QSCALE = 1.4426950408889634 / 128.0   # log2e / 128 folded into Wq
ACT_EXP_SCALE = 128.0 * 0.6931471805599453  # recovers exp(E) on ScalarE

F32 = mybir.dt.float32
F32R = mybir.dt.float32r
BF16 = mybir.dt.bfloat16

T = 3072          # sequence length (and t2 size)
D = 1024          # model dim
H = 16            # heads
NCORE = 8
EG = 64           # channel groups per head (columns of Y_h)
XI = 64           # "feature" dim of the quirky attention (t // 48)
NKT = D // 128    # 8 contraction tiles for the projection
NTB = T // 128    # 24 t-blocks / c-tiles
RCH = 512         # r-chunk (free dim of energy/AV matmuls)
NR = T // RCH     # 6 r-chunks
W6 = 6 * EG       # 384 fused projection output columns
# c-tiles whose softmax exp runs on the Vector engine (Schraudolph bit-trick
# via one tensor_scalar) instead of ScalarE -- splits the softmax exp load.
DVE_SET = frozenset({1, 3, 5, 7, 9, 11, 13, 15, 17, 19, 21})
SCHRAUD_A = float(2.0 ** 14)          # 2^7 * 128 (E' is prescaled by 1/128)
SCHRAUD_B = 16250.368                 # (127 - 0.044) * 2^7


def _split_multiwaits(nc):
    """This toolchain's walrus accepts at most ONE sync wait per
    instruction (setupSyncWait: 'Too many sync wait commands'), but Tile
    attaches several. Hoist all but the last wait of each instruction onto
    same-engine NoOps inserted right before it — semantically identical
    (sem-ge waits executed in sequence)."""
    n = 0
    for fn in nc.m.functions:
        for bb in fn.blocks:
            insts = list(bb.instructions)
            out = []
            changed = False
            for inst in insts:
                si = inst.sync_info
                if si is not None and len(si.on_wait) > 1:
                    waits = list(si.on_wait)
                    for w in waits[:-1]:
                        n += 1
                        out.append(mybir.InstNoOp(
                            name=f"I-splitwait-{n}",
                            ins=[], outs=[], engine=inst.engine,
                            sync_info=mybir.SyncInfo(on_wait=[w], on_update=[]),
                        ))
                    inst.sync_info = mybir.SyncInfo(
                        on_wait=[waits[-1]], on_update=list(si.on_update)
                    )
                    changed = True
                out.append(inst)
            if changed:
                bb.instructions = out
    return n


def _coarsen_sem_incs(nc):
    """Tile attaches a sem-inc to EVERY instruction (its optimize_sems pass
    is disabled).  Each inc is a serializing EVT_SEM register write (~26ns)
    and breaks PE row-group matmul pairing.  Drop incs whose cumulative
    values no wait ever references, folding the dropped counts into the next
    kept inc -- sem values at every waited point are unchanged.  DMA sems
    (completion-fired) are left untouched."""
    import collections

    waitvals = collections.defaultdict(set)
    badsem = set()
    for fn in nc.m.functions:
        for bb in fn.blocks:
            for inst in bb.instructions:
                si = inst.sync_info
                if si is None:
                    continue
                for w in si.on_wait:
                    if (
                        w.sync_type != "semaphore"
                        or w.wait_mode != "sem-ge-imm"
                        or w.wait_reg is not None
                    ):
                        badsem.add(w.id)
                    else:
                        waitvals[w.id].add(w.wait_value)

    upd_sites = collections.defaultdict(list)
    for fn in nc.m.functions:
        for bb in fn.blocks:
            for inst in bb.instructions:
                si = inst.sync_info
                if si is None:
                    continue
                for u in si.on_update:
                    if (
                        u.sync_type != "semaphore"
                        or u.update_mode != "sem-inc"
                        or u.update_reg is not None
                        or (u.ant_name or "").startswith("DMA")
                    ):
                        badsem.add(u.id)
                        continue
                    upd_sites[u.id].append((id(bb), inst, u))

    for sid, sites in upd_sites.items():
        if len({b for b, _, _ in sites}) > 1:
            badsem.add(sid)
        if len({inst.engine for _, inst, _ in sites}) > 1:
            badsem.add(sid)

    # walrus requires every sem-inc to have update_value == 1, so dropped
    # incs cannot be folded into a bigger one.  Instead drop them outright
    # and renumber every wait on that sem to the new (smaller) counts.
    ndrop = 0
    dropset = set()          # ids of SyncUpdate objects to drop
    remap = {}               # sem id -> sorted list of kept cum values
    for sid, sites in upd_sites.items():
        if sid in badsem:
            continue
        wv = waitvals.get(sid, set())
        cum = 0
        kept_cums = []
        for i, (_, inst, u) in enumerate(sites):
            cum += u.update_value
            if (cum in wv) or (i == len(sites) - 1):
                kept_cums.append(cum)
            else:
                dropset.add(id(u))
                ndrop += 1
        remap[sid] = kept_cums

    import bisect

    for fn in nc.m.functions:
        for bb in fn.blocks:
            for inst in bb.instructions:
                si = inst.sync_info
                if si is None:
                    continue
                new_upd = [u for u in si.on_update if id(u) not in dropset]
                new_wait = []
                for w in si.on_wait:
                    if w.id in remap:
                        kept = remap[w.id]
                        # new value = rank of first kept cum >= old value
                        nv = bisect.bisect_left(kept, w.wait_value) + 1
                        nv = min(nv, len(kept))
                        if nv != w.wait_value:
                            w = mybir.SyncWait(
                                sync_type=w.sync_type,
                                id=w.id,
                                ant_name=w.ant_name,
                                wait_mode=w.wait_mode,
                                wait_value=nv,
                                wait_reg=w.wait_reg,
                            )
                    new_wait.append(w)
                if len(new_upd) != len(si.on_update) or any(
                    a is not b for a, b in zip(new_wait, si.on_wait)
                ):
                    inst.sync_info = mybir.SyncInfo(
                        on_wait=new_wait, on_update=new_upd
                    )
    return ndrop


def build_program():
    nc = bass.Bass()

    xT = nc.dram_tensor("xT", [NTB, 128, NKT, 128], BF16, kind="ExternalInput")
    w6 = nc.dram_tensor("w6", [D, W6], BF16, kind="ExternalInput")
    b6 = nc.dram_tensor("b6", [128, W6], F32, kind="ExternalInput")
    y6qk = nc.dram_tensor("y6qk", [4, T, EG], BF16, kind="Internal")
    y6v = nc.dram_tensor("y6v", [2, T, EG], BF16, kind="Internal")
    outT = nc.dram_tensor("outT", [2, XI + 1, T], F32, kind="ExternalOutput")

    with tile.TileContext(nc) as tc:
        with tc.tile_pool(name="const", bufs=1) as constp:
            w6_sb = constp.tile([128, NKT, W6], BF16)
            w6v = w6[:, :].rearrange("(k p) n -> k p n", p=128)
            for k in range(NKT):
                nc.scalar.dma_start(out=w6_sb[:, k, :], in_=w6v[k, :, :])
            b6_sb = constp.tile([128, W6], F32)
            nc.scalar.dma_start(out=b6_sb, in_=b6[:, :])
            # identity blocks at partitions 0:64 and 64:128 so the two heads'
            # V^T transposes run row-paired in the PE array
            ident = constp.tile([128, 64], BF16)
            nc.gpsimd.memset(ident, 0.0)
            make_identity(nc, ident[0:64, :], nomemset=True)
            make_identity(nc, ident[64:128, :], nomemset=True)
            ones_f32 = constp.tile([128, 1], F32)
            nc.gpsimd.memset(ones_f32, 1.0)
            kt_sb = constp.tile([128, T], BF16)   # rows 0:64 h1 K^T, 64:128 h2
            qt_sb = constp.tile([128, T], BF16)   # rows 0:64 h1 Q^T, 64:128 h2
            vt_sb = constp.tile([128, T], BF16)   # rows 0:64 h1 V^T, 64:128 h2
            # V tiles augmented with a ones column: [:, c, 0:64] = V_h c-tile,
            # [:, c, 64] = 1.0 so one matmul computes out^T AND Sigma (row 64)
            v1_sb = constp.tile([128, NTB, XI + 1], BF16)
            v2_sb = constp.tile([128, NTB, XI + 1], BF16)
            for vsb in (v1_sb, v2_sb):
                for c in range(NTB):
                    nc.vector.tensor_copy(vsb[:, c, XI:XI + 1], ones_f32)

            # ---------------- projection: Y6 = X @ W6^T + b6 ----------------
            with tc.tile_pool(name="xt", bufs=6) as xtp, \
                 tc.tile_pool(name="psy", bufs=4, space="PSUM") as psyp, \
                 tc.tile_pool(name="ysb", bufs=4) as ysbp:
                for j in range(NTB):
                    xt = xtp.tile([128, NKT, 128], BF16)
                    (nc.sync if j % 2 == 0 else nc.scalar).dma_start(
                        out=xt.rearrange("p k t -> p (k t)"),
                        in_=xT[j, :, :, :].rearrange("p k t -> p (k t)"),
                    )
                    psy = psyp.tile([128, W6], F32)
                    for k in range(NKT):
                        nc.tensor.matmul(
                            psy, xt[:, k, :], w6_sb[:, k, :],
                            start=(k == 0), stop=(k == NKT - 1),
                        )
                    psyv = psy.rearrange("p (h z e) -> p h z e", h=2, z=3)
                    b6v = b6_sb.rearrange("p (h z e) -> p h z e", h=2, z=3)
                    ysbqk = ysbp.tile([128, 2, 2, EG], BF16, name="ysbqk")
                    nc.vector.tensor_add(ysbqk, psyv[:, :, 0:2, :],
                                         b6v[:, :, 0:2, :])
                    ysbv = ysbp.tile([128, 2, EG], BF16, name="ysbv")
                    nc.vector.tensor_add(ysbv, psyv[:, :, 2, :],
                                         b6v[:, :, 2, :])
                    nc.scalar.dma_start(
                        out=y6qk[:, ts(j, 128), :].rearrange("q t e -> t q e"),
                        in_=ysbqk,
                    )
                    nc.sync.dma_start(
                        out=y6v[:, ts(j, 128), :].rearrange("q t e -> t q e"),
                        in_=ysbv,
                    )

            # ------- load Q^T / K^T / V^T as contiguous (64, 3072) views.
            # Flat 2D APs -> one 6KB descriptor per partition (NOT 48x128B).
            engs = (nc.sync, nc.scalar)
            ldn = 0
            for srcap, bufap in (
                    (y6v[0, :, :], vt_sb[0:64, :]),
                    (y6v[1, :, :], vt_sb[64:128, :]),
                    (y6qk[1, :, :], kt_sb[0:64, :]),
                    (y6qk[3, :, :], kt_sb[64:128, :]),
                    (y6qk[0, :, :], qt_sb[0:64, :]),
                    (y6qk[2, :, :], qt_sb[64:128, :])):
                engs[ldn % 2].dma_start(
                    out=bufap,
                    in_=srcap.rearrange("(xi a) e -> xi (a e)", xi=64),
                )
                ldn += 1

            # ------- V tiles: true transpose of V^T chunks via the PE -------
            # 4 c-tiles batched per PSUM tile so one DVE copy moves 4 blocks
            with tc.tile_pool(name="vtps", bufs=4, space="PSUM") as vtpsp:
                for c0 in range(0, NTB, 4):
                    for vsb, row0 in ((v1_sb, 0), (v2_sb, 64)):
                        vp = vtpsp.tile([128, 4, XI], BF16)
                        for i in range(4):
                            nc.tensor.transpose(
                                vp[:, i, :],
                                vt_sb[row0:row0 + 64, ts(c0 + i, 128)],
                                ident[row0:row0 + 64, :],
                            )
                        nc.vector.tensor_copy(
                            vsb[:, ds(c0, 4), 0:XI], vp
                        )

            # --------------------------- attention --------------------------
            # Per c-tile the two heads' energy matmuls are emitted back to
            # back into the same PSUM tile (disjoint PE row groups 0:64 /
            # 64:128 -> the array runs them concurrently).  Each (128, 1024)
            # energy pair is exponentiated on ScalarE (exp) OR on the Vector
            # engine (custom poly^128 ops) per DVE_SET to split the softmax
            # exp across both engines.
            with tc.tile_pool(name="eps", bufs=3, space="PSUM") as epp, \
                 tc.tile_pool(name="ex", bufs=8) as expool, \
                 tc.tile_pool(name="outp", bufs=1, space="PSUM") as outpp, \
                 tc.tile_pool(name="osb", bufs=4) as osbp:
                for r in range(NR):
                    qt = qt_sb[:, ts(r, RCH)]
                    outp1 = outpp.tile([XI + 1, RCH], F32)
                    outp2 = outpp.tile([XI + 1, RCH], F32)
                    for c in range(NTB):
                        ep = epp.tile([128, 2, RCH], F32)
                        nc.tensor.matmul(
                            ep[:, 0, :], kt_sb[0:64, ts(c, 128)],
                            qt[0:64, :], start=True, stop=True,
                        )
                        nc.tensor.matmul(
                            ep[:, 1, :], kt_sb[64:128, ts(c, 128)],
                            qt[64:128, :], start=True, stop=True,
                        )
                        ex = expool.tile([128, 2, RCH], BF16)
                        if c in DVE_SET:
                            # Schraudolph exp on the DVE: one tensor_scalar
                            # computes i16(round(E'*2^14 + B)); the int bits
                            # ARE the bf16 approximation of exp(E).
                            nc.vector.tensor_scalar(
                                ex.bitcast(mybir.dt.int16), ep,
                                SCHRAUD_A, SCHRAUD_B,
                                mybir.AluOpType.mult, mybir.AluOpType.add,
                            )
                        else:
                            nc.scalar.activation(
                                ex, ep, mybir.ActivationFunctionType.Exp,
                                scale=ACT_EXP_SCALE,
                            )
                        nc.tensor.matmul(
                            outp1, v1_sb[:, c, :], ex[:, 0, :],
                            start=(c == 0), stop=(c == NTB - 1),
                        )
                        nc.tensor.matmul(
                            outp2, v2_sb[:, c, :], ex[:, 1, :],
                            start=(c == 0), stop=(c == NTB - 1),
                        )
                    for outp, hl in ((outp1, 0), (outp2, 1)):
                        osb = osbp.tile([XI + 1, RCH], F32)
                        # split the PSUM drain across both engines so the
                        # bank frees ~2x sooner for the next r-chunk
                        nc.vector.tensor_copy(
                            osb[:, 0:RCH // 2], outp[:, 0:RCH // 2]
                        )
                        nc.scalar.copy(
                            osb[:, RCH // 2:RCH], outp[:, RCH // 2:RCH]
                        )
                        nc.gpsimd.dma_start(
                            out=outT[hl, :, ts(r, RCH)], in_=osb
                        )
    return nc


def make_in_maps(x, Wq, bq, Wk, bk, Wv, bv):
    X = np.ascontiguousarray(np.asarray(x, dtype=np.float32).reshape(T, D))
    # (NTB, 128, NKT, 128): [j, p, k, t] = X[128j+t, 128k+p] -- every SBUF
    # partition reads one contiguous 2KB run per projection slab DMA
    xTm = np.ascontiguousarray(
        X.astype(BF16NP).reshape(NTB, 128, NKT, 128).transpose(0, 3, 2, 1)
    )
    in_maps = []
    for c in range(NCORE):
        wcols, bcols = [], []
        for h in (2 * c, 2 * c + 1):
            for W, b, s in ((Wq, bq, QSCALE), (Wk, bk, 1.0), (Wv, bv, 1.0)):
                wcols.append(np.asarray(W, np.float32)[h::H, :].T * np.float32(s))
                bcols.append(np.asarray(b, np.float32)[h::H] * np.float32(s))
        w6m = np.ascontiguousarray(np.concatenate(wcols, axis=1).astype(BF16NP))
        b6m = np.ascontiguousarray(
            np.broadcast_to(np.concatenate(bcols), (128, W6)).astype(np.float32)
        )
        in_maps.append({"xT": xTm, "w6": w6m, "b6": b6m})
    return X, in_maps


def assemble(X, results, gamma):
    O = np.empty((T, EG, H), dtype=np.float32)
    for c in range(NCORE):
        res = results[c]
        for hl in range(2):
            h = 2 * c + hl
            onn = res["outT"][hl][0:XI, :]                # (64, 3072)
            s = res["outT"][hl][XI, :]                    # (3072,)
            O[:, :, h] = (onn / s[None, :]).T
    out = O.reshape(T, D)
    g = np.float32(np.asarray(gamma))
    return (g * out + X).reshape(1, 1, T, D).astype(np.float32)


_PROGRAM = None
last_run_info = {}


def kernel(x, Wq, bq, Wk, bk, Wv, bv, gamma):
    global _PROGRAM
    from concourse import bass_utils

    X, in_maps = make_in_maps(x, Wq, bq, Wk, bk, Wv, bv)
    if _PROGRAM is None:
        import os

        _PROGRAM = build_program()
        if os.environ.get("COARSEN") == "1":
            _coarsen_sem_incs(_PROGRAM)
        # required for this toolchain's walrus (1 sync wait per instruction);
        # applied here so CoreSim (which predates these NoOps) can still run
        # the unsplit program from build_program()
        _split_multiwaits(_PROGRAM)
    res = bass_utils.run_bass_kernel_spmd(
        _PROGRAM, in_maps, core_ids=list(range(NCORE))
    )
    last_run_info["exec_time_ns"] = res.exec_time_ns
    last_run_info["trace"] = res.instructions_and_trace
    return assemble(X, res.results, gamma)



# revision 27
# speedup vs baseline: 1.0427x; 1.0333x over previous
"""Trainium2 Bass kernel for nn_MultiHeadAttention_68865505624655.

Strategy (head parallelism, 8 cores x 2 heads, bf16 pipeline):
  The reference's reshape(B,-1,T,H) mixes time/channel dims. For head h the
  per-head matrices are exactly reinterpretations of the compacted projection
  output Y_h = X @ W[h::16].T (shape (3072, 64)):
      Q_h^T (xi, t2)  == Y_h viewed as (64, 3072)   (same linear memory!)
      K_h^T (xi, t2)  == same
      V_h  (t2', xi)  == transpose of that view     (needs a real transpose)
  log2e/128 is folded into Wq host-side, so the "energy" the PE produces is
  E' = E * log2e/128 -- the form both exp paths below want.
  Each core:
    1. fused QKV projection for its 2 heads in bf16: Y6 = X @ W6^T + b
       (24 t-blocks x 8 k-tiles), written to DRAM scratch (bf16 planes).
    2. reads back Q^T/K^T/V^T as contiguous (64,3072) views (one 6KB
       descriptor per partition); V tiles via PE transposes (4 per DVE copy).
    3. per c-tile, the two heads' energy matmuls (K=64) go back-to-back into
       one PSUM tile at tile_position (0,0)/(64,0) -> the PE runs the pair
       concurrently in disjoint row groups (~1.75x).  exp(E) alternates
       between ScalarE (Exp activation, scale=128*ln2) and the Vector engine
       (Schraudolph bit-trick: ONE tensor_scalar computing
       int16(round(E'*2^14 + B)) whose bits are the bf16 approximation of
       exp(E), ~3% max rel err) so neither engine is the softmax bottleneck.
       One bf16 matmul per (c,head) with lhsT = [V_c | 1] (M=65) accumulates
       BOTH out^T[xi,r] and the softmax denominator Sigma[r] (row 64).
    4. writes per-head [out^T; Sigma] (65,3072) tiles per core.
  Host: divide rows 0:64 by row 64, interleave heads into (T,D), gamma*out+x.
  Toolchain workarounds: _split_multiwaits (this walrus allows one sync wait
  per instruction) and _install_ntff_shim (axon NTFF profiling hook).
"""

import sys

if "/opt/trn_rl_repo" not in sys.path:
    sys.path.insert(0, "/opt/trn_rl_repo")

import ml_dtypes
import numpy as np

BF16NP = ml_dtypes.bfloat16


def _install_ntff_shim():
    """concourse.bass_utils under axon imports antenv.axon_hooks when
    tracing is requested; this image's antenv lacks that submodule.
    Register an equivalent shim (backed by the boot image's ctypes NTFF
    driver) so BASS_TRACE=1 profiles instead of crashing."""
    import types

    if "antenv.axon_hooks" in sys.modules:
        return
    mod = types.ModuleType("antenv.axon_hooks")
    cell = {}

    def get_axon_ntff_profile_hook():
        if "h" not in cell:
            try:
                from trn_agent_boot.trn_boot import _ntff_profile_via_ctypes
                cell["h"] = _ntff_profile_via_ctypes("/opt/axon/libaxon_pjrt.so")
            except Exception:
                cell["h"] = None
        return cell["h"]

    def set_axon_ntff_profile_hook(h):
        cell["h"] = h

    mod.get_axon_ntff_profile_hook = get_axon_ntff_profile_hook
    mod.set_axon_ntff_profile_hook = set_axon_ntff_profile_hook
    sys.modules["antenv.axon_hooks"] = mod


_install_ntff_shim()

import concourse.bass as bass
import concourse.mybir as mybir
import concourse.tile as tile
from concourse.bass import ds, ts
from concourse.masks import make_identity

QSCALE = 1.4426950408889634 / 128.0   # log2e / 128 folded into Wq
ACT_EXP_SCALE = 128.0 * 0.6931471805599453  # recovers exp(E) on ScalarE

F32 = mybir.dt.float32
F32R = mybir.dt.float32r
BF16 = mybir.dt.bfloat16

T = 3072          # sequence length (and t2 size)
D = 1024          # model dim
H = 16            # heads
NCORE = 8
EG = 64           # channel groups per head (columns of Y_h)
XI = 64           # "feature" dim of the quirky attention (t // 48)
NKT = D // 128    # 8 contraction tiles for the projection
NTB = T // 128    # 24 t-blocks / c-tiles
RCH = 512         # r-chunk (free dim of energy/AV matmuls)
NR = T // RCH     # 6 r-chunks
W6 = 6 * EG       # 384 fused projection output columns
# c-tiles whose softmax exp runs on the Vector engine (Schraudolph bit-trick
# via one tensor_scalar) instead of ScalarE -- splits the softmax exp load.
DVE_SET = frozenset({1, 3, 5, 7, 9, 11, 13, 15, 17, 19, 21})
SCHRAUD_A = float(2.0 ** 14)          # 2^7 * 128 (E' is prescaled by 1/128)
SCHRAUD_B = 16250.368                 # (127 - 0.044) * 2^7


def _split_multiwaits(nc):
    """This toolchain's walrus accepts at most ONE sync wait per
    instruction (setupSyncWait: 'Too many sync wait commands'), but Tile
    attaches several. Hoist all but the last wait of each instruction onto
    same-engine NoOps inserted right before it — semantically identical
    (sem-ge waits executed in sequence)."""
    n = 0
    for fn in nc.m.functions:
        for bb in fn.blocks:
            insts = list(bb.instructions)
            out = []
            changed = False
            for inst in insts:
                si = inst.sync_info
                if si is not None and len(si.on_wait) > 1:
                    waits = list(si.on_wait)
                    for w in waits[:-1]:
                        n += 1
                        out.append(mybir.InstNoOp(
                            name=f"I-splitwait-{n}",
                            ins=[], outs=[], engine=inst.engine,
                            sync_info=mybir.SyncInfo(on_wait=[w], on_update=[]),
                        ))
                    inst.sync_info = mybir.SyncInfo(
                        on_wait=[waits[-1]], on_update=list(si.on_update)
                    )
                    changed = True
                out.append(inst)
            if changed:
                bb.instructions = out
    return n


def _coarsen_sem_incs(nc):
    """Tile attaches a sem-inc to EVERY instruction (its optimize_sems pass
    is disabled).  Each inc is a serializing EVT_SEM register write (~26ns)
    and breaks PE row-group matmul pairing.  Drop incs whose cumulative
    values no wait ever references, folding the dropped counts into the next
    kept inc -- sem values at every waited point are unchanged.  DMA sems
    (completion-fired) are left untouched."""
    import collections

    waitvals = collections.defaultdict(set)
    badsem = set()
    for fn in nc.m.functions:
        for bb in fn.blocks:
            for inst in bb.instructions:
                si = inst.sync_info
                if si is None:
                    continue
                for w in si.on_wait:
                    if (
                        w.sync_type != "semaphore"
                        or w.wait_mode != "sem-ge-imm"
                        or w.wait_reg is not None
                    ):
                        badsem.add(w.id)
                    else:
                        waitvals[w.id].add(w.wait_value)

    upd_sites = collections.defaultdict(list)
    for fn in nc.m.functions:
        for bb in fn.blocks:
            for inst in bb.instructions:
                si = inst.sync_info
                if si is None:
                    continue
                for u in si.on_update:
                    if (
                        u.sync_type != "semaphore"
                        or u.update_mode != "sem-inc"
                        or u.update_reg is not None
                        or (u.ant_name or "").startswith("DMA")
                    ):
                        badsem.add(u.id)
                        continue
                    upd_sites[u.id].append((id(bb), inst, u))

    for sid, sites in upd_sites.items():
        if len({b for b, _, _ in sites}) > 1:
            badsem.add(sid)
        if len({inst.engine for _, inst, _ in sites}) > 1:
            badsem.add(sid)

    # walrus requires every sem-inc to have update_value == 1, so dropped
    # incs cannot be folded into a bigger one.  Instead drop them outright
    # and renumber every wait on that sem to the new (smaller) counts.
    ndrop = 0
    dropset = set()          # ids of SyncUpdate objects to drop
    remap = {}               # sem id -> sorted list of kept cum values
    for sid, sites in upd_sites.items():
        if sid in badsem:
            continue
        wv = waitvals.get(sid, set())
        cum = 0
        kept_cums = []
        for i, (_, inst, u) in enumerate(sites):
            cum += u.update_value
            if (cum in wv) or (i == len(sites) - 1):
                kept_cums.append(cum)
            else:
                dropset.add(id(u))
                ndrop += 1
        remap[sid] = kept_cums

    import bisect

    for fn in nc.m.functions:
        for bb in fn.blocks:
            for inst in bb.instructions:
                si = inst.sync_info
                if si is None:
                    continue
                new_upd = [u for u in si.on_update if id(u) not in dropset]
                new_wait = []
                for w in si.on_wait:
                    if w.id in remap:
                        kept = remap[w.id]
                        # new value = rank of first kept cum >= old value
                        nv = bisect.bisect_left(kept, w.wait_value) + 1
                        nv = min(nv, len(kept))
                        if nv != w.wait_value:
                            w = mybir.SyncWait(
                                sync_type=w.sync_type,
                                id=w.id,
                                ant_name=w.ant_name,
                                wait_mode=w.wait_mode,
                                wait_value=nv,
                                wait_reg=w.wait_reg,
                            )
                    new_wait.append(w)
                if len(new_upd) != len(si.on_update) or any(
                    a is not b for a, b in zip(new_wait, si.on_wait)
                ):
                    inst.sync_info = mybir.SyncInfo(
                        on_wait=new_wait, on_update=new_upd
                    )
    return ndrop


def build_program():
    nc = bass.Bass()

    xT = nc.dram_tensor("xT", [NTB, 128, NKT, 128], BF16, kind="ExternalInput")
    w6 = nc.dram_tensor("w6", [D, W6], BF16, kind="ExternalInput")
    b6 = nc.dram_tensor("b6", [128, W6], F32, kind="ExternalInput")
    y6qk = nc.dram_tensor("y6qk", [4, T, EG], BF16, kind="Internal")
    y6v = nc.dram_tensor("y6v", [2, T, EG], BF16, kind="Internal")
    outT = nc.dram_tensor("outT", [2, XI + 1, T], F32, kind="ExternalOutput")

    with tile.TileContext(nc) as tc:
        with tc.tile_pool(name="const", bufs=1) as constp:
            w6_sb = constp.tile([128, NKT, W6], BF16)
            w6v = w6[:, :].rearrange("(k p) n -> k p n", p=128)
            for k in range(NKT):
                nc.scalar.dma_start(out=w6_sb[:, k, :], in_=w6v[k, :, :])
            b6_sb = constp.tile([128, W6], F32)
            nc.scalar.dma_start(out=b6_sb, in_=b6[:, :])
            # identity blocks at partitions 0:64 and 64:128 so the two heads'
            # V^T transposes run row-paired in the PE array
            ident = constp.tile([128, 64], BF16)
            nc.gpsimd.memset(ident, 0.0)
            make_identity(nc, ident[0:64, :], nomemset=True)
            make_identity(nc, ident[64:128, :], nomemset=True)
            ones_f32 = constp.tile([128, 1], F32)
            nc.gpsimd.memset(ones_f32, 1.0)
            kt_sb = constp.tile([128, T], BF16)   # rows 0:64 h1 K^T, 64:128 h2
            qt_sb = constp.tile([128, T], BF16)   # rows 0:64 h1 Q^T, 64:128 h2
            vt_sb = constp.tile([128, T], BF16)   # rows 0:64 h1 V^T, 64:128 h2
            # V tiles augmented with a ones column: [:, c, 0:64] = V_h c-tile,
            # [:, c, 64] = 1.0 so one matmul computes out^T AND Sigma (row 64)
            v1_sb = constp.tile([128, NTB, XI + 1], BF16)
            v2_sb = constp.tile([128, NTB, XI + 1], BF16)
            for vsb in (v1_sb, v2_sb):
                for c in range(NTB):
                    nc.vector.tensor_copy(vsb[:, c, XI:XI + 1], ones_f32)

            # ---------------- projection: Y6 = X @ W6^T + b6 ----------------
            with tc.tile_pool(name="xt", bufs=6) as xtp, \
                 tc.tile_pool(name="psy", bufs=4, space="PSUM") as psyp, \
                 tc.tile_pool(name="ysb", bufs=4) as ysbp:
                for j in range(NTB):
                    xt = xtp.tile([128, NKT, 128], BF16)
                    (nc.sync if j % 2 == 0 else nc.scalar).dma_start(
                        out=xt.rearrange("p k t -> p (k t)"),
                        in_=xT[j, :, :, :].rearrange("p k t -> p (k t)"),
                    )
                    psy = psyp.tile([128, W6], F32)
                    for k in range(NKT):
                        nc.tensor.matmul(
                            psy, xt[:, k, :], w6_sb[:, k, :],
                            start=(k == 0), stop=(k == NKT - 1),
                        )
                    psyv = psy.rearrange("p (h z e) -> p h z e", h=2, z=3)
                    b6v = b6_sb.rearrange("p (h z e) -> p h z e", h=2, z=3)
                    ysbqk = ysbp.tile([128, 2, 2, EG], BF16, name="ysbqk")
                    nc.vector.tensor_add(ysbqk, psyv[:, :, 0:2, :],
                                         b6v[:, :, 0:2, :])
                    ysbv = ysbp.tile([128, 2, EG], BF16, name="ysbv")
                    nc.vector.tensor_add(ysbv, psyv[:, :, 2, :],
                                         b6v[:, :, 2, :])
                    nc.scalar.dma_start(
                        out=y6qk[:, ts(j, 128), :].rearrange("q t e -> t q e"),
                        in_=ysbqk,
                    )
                    nc.sync.dma_start(
                        out=y6v[:, ts(j, 128), :].rearrange("q t e -> t q e"),
                        in_=ysbv,
                    )

            # ------- load Q^T / K^T / V^T as contiguous (64, 3072) views.
            # Flat 2D APs -> one 6KB descriptor per partition (NOT 48x128B).
            engs = (nc.sync, nc.scalar)
            ldn = 0
            for srcap, bufap in (
                    (y6v[0, :, :], vt_sb[0:64, :]),
                    (y6v[1, :, :], vt_sb[64:128, :]),
                    (y6qk[1, :, :], kt_sb[0:64, :]),
                    (y6qk[3, :, :], kt_sb[64:128, :]),
                    (y6qk[0, :, :], qt_sb[0:64, :]),
                    (y6qk[2, :, :], qt_sb[64:128, :])):
                engs[ldn % 2].dma_start(
                    out=bufap,
                    in_=srcap.rearrange("(xi a) e -> xi (a e)", xi=64),
                )
                ldn += 1

            # ------- V tiles: true transpose of V^T chunks via the PE -------
            # 4 c-tiles batched per PSUM tile so one DVE copy moves 4 blocks
            with tc.tile_pool(name="vtps", bufs=4, space="PSUM") as vtpsp:
                for c0 in range(0, NTB, 4):
                    for vsb, row0 in ((v1_sb, 0), (v2_sb, 64)):
                        vp = vtpsp.tile([128, 4, XI], BF16)
                        for i in range(4):
                            nc.tensor.transpose(
                                vp[:, i, :],
                                vt_sb[row0:row0 + 64, ts(c0 + i, 128)],
                                ident[row0:row0 + 64, :],
                            )
                        nc.vector.tensor_copy(
                            vsb[:, ds(c0, 4), 0:XI], vp
                        )

            # --------------------------- attention --------------------------
            # Per c-tile the two heads' energy matmuls are emitted back to
            # back into the same PSUM tile (disjoint PE row groups 0:64 /
            # 64:128 -> the array runs them concurrently).  Each (128, 1024)
            # energy pair is exponentiated on ScalarE (exp) OR on the Vector
            # engine (custom poly^128 ops) per DVE_SET to split the softmax
            # exp across both engines.
            with tc.tile_pool(name="eps", bufs=3, space="PSUM") as epp, \
                 tc.tile_pool(name="ex", bufs=8) as expool, \
                 tc.tile_pool(name="outp", bufs=1, space="PSUM") as outpp, \
                 tc.tile_pool(name="osb", bufs=4) as osbp:
                for r in range(NR):
                    qt = qt_sb[:, ts(r, RCH)]
                    outp1 = outpp.tile([XI + 1, RCH], F32)
                    outp2 = outpp.tile([XI + 1, RCH], F32)
                    LAG = 3
                    exq = {}
                    for cc in range(NTB + LAG):
                        if cc < NTB:
                            c = cc
                            ep = epp.tile([128, 2, RCH], F32)
                            nc.tensor.matmul(
                                ep[:, 0, :], kt_sb[0:64, ts(c, 128)],
                                qt[0:64, :], start=True, stop=True,
                            )
                            nc.tensor.matmul(
                                ep[:, 1, :], kt_sb[64:128, ts(c, 128)],
                                qt[64:128, :], start=True, stop=True,
                            )
                            ex = expool.tile([128, 2, RCH], BF16)
                            if c in DVE_SET:
                                nc.vector.tensor_scalar(
                                    ex.bitcast(mybir.dt.int16), ep,
                                    SCHRAUD_A, SCHRAUD_B,
                                    mybir.AluOpType.mult, mybir.AluOpType.add,
                                )
                            else:
                                nc.scalar.activation(
                                    ex, ep, mybir.ActivationFunctionType.Exp,
                                    scale=ACT_EXP_SCALE,
                                )
                            exq[c] = ex
                        if cc >= LAG:
                            # AV lags the energy/exp stream by LAG c-tiles so
                            # the exp engines always lead the PE's consumption
                            c = cc - LAG
                            ex = exq.pop(c)
                            nc.tensor.matmul(
                                outp1, v1_sb[:, c, :], ex[:, 0, :],
                                start=(c == 0), stop=(c == NTB - 1),
                            )
                            nc.tensor.matmul(
                                outp2, v2_sb[:, c, :], ex[:, 1, :],
                                start=(c == 0), stop=(c == NTB - 1),
                            )
                    for outp, hl in ((outp1, 0), (outp2, 1)):
                        osb = osbp.tile([XI + 1, RCH], F32)
                        # split the PSUM drain across both engines so the
                        # bank frees ~2x sooner for the next r-chunk
                        nc.vector.tensor_copy(
                            osb[:, 0:RCH // 2], outp[:, 0:RCH // 2]
                        )
                        nc.scalar.copy(
                            osb[:, RCH // 2:RCH], outp[:, RCH // 2:RCH]
                        )
                        nc.gpsimd.dma_start(
                            out=outT[hl, :, ts(r, RCH)], in_=osb
                        )
    return nc


def make_in_maps(x, Wq, bq, Wk, bk, Wv, bv):
    X = np.ascontiguousarray(np.asarray(x, dtype=np.float32).reshape(T, D))
    # (NTB, 128, NKT, 128): [j, p, k, t] = X[128j+t, 128k+p] -- every SBUF
    # partition reads one contiguous 2KB run per projection slab DMA
    xTm = np.ascontiguousarray(
        X.astype(BF16NP).reshape(NTB, 128, NKT, 128).transpose(0, 3, 2, 1)
    )
    in_maps = []
    for c in range(NCORE):
        wcols, bcols = [], []
        for h in (2 * c, 2 * c + 1):
            for W, b, s in ((Wq, bq, QSCALE), (Wk, bk, 1.0), (Wv, bv, 1.0)):
                wcols.append(np.asarray(W, np.float32)[h::H, :].T * np.float32(s))
                bcols.append(np.asarray(b, np.float32)[h::H] * np.float32(s))
        w6m = np.ascontiguousarray(np.concatenate(wcols, axis=1).astype(BF16NP))
        b6m = np.ascontiguousarray(
            np.broadcast_to(np.concatenate(bcols), (128, W6)).astype(np.float32)
        )
        in_maps.append({"xT": xTm, "w6": w6m, "b6": b6m})
    return X, in_maps


def assemble(X, results, gamma):
    O = np.empty((T, EG, H), dtype=np.float32)
    for c in range(NCORE):
        res = results[c]
        for hl in range(2):
            h = 2 * c + hl
            onn = res["outT"][hl][0:XI, :]                # (64, 3072)
            s = res["outT"][hl][XI, :]                    # (3072,)
            O[:, :, h] = (onn / s[None, :]).T
    out = O.reshape(T, D)
    g = np.float32(np.asarray(gamma))
    return (g * out + X).reshape(1, 1, T, D).astype(np.float32)


_PROGRAM = None
last_run_info = {}


def kernel(x, Wq, bq, Wk, bk, Wv, bv, gamma):
    global _PROGRAM
    from concourse import bass_utils

    X, in_maps = make_in_maps(x, Wq, bq, Wk, bk, Wv, bv)
    if _PROGRAM is None:
        import os

        _PROGRAM = build_program()
        if os.environ.get("COARSEN") == "1":
            _coarsen_sem_incs(_PROGRAM)
        # required for this toolchain's walrus (1 sync wait per instruction);
        # applied here so CoreSim (which predates these NoOps) can still run
        # the unsplit program from build_program()
        _split_multiwaits(_PROGRAM)
    res = bass_utils.run_bass_kernel_spmd(
        _PROGRAM, in_maps, core_ids=list(range(NCORE))
    )
    last_run_info["exec_time_ns"] = res.exec_time_ns
    last_run_info["trace"] = res.instructions_and_trace
    return assemble(X, res.results, gamma)



# revision 28
# speedup vs baseline: 1.0776x; 1.0335x over previous
"""Trainium2 Bass kernel for nn_MultiHeadAttention_68865505624655.

Strategy (head parallelism, 8 cores x 2 heads, bf16 pipeline):
  The reference's reshape(B,-1,T,H) mixes time/channel dims. For head h the
  per-head matrices are exactly reinterpretations of the compacted projection
  output Y_h = X @ W[h::16].T (shape (3072, 64)):
      Q_h^T (xi, t2)  == Y_h viewed as (64, 3072)   (same linear memory!)
      K_h^T (xi, t2)  == same
      V_h  (t2', xi)  == transpose of that view     (needs a real transpose)
  log2e/128 is folded into Wq host-side, so the "energy" the PE produces is
  E' = E * log2e/128 -- the form both exp paths below want.
  Each core:
    1. fused QKV projection for its 2 heads in bf16: Y6 = X @ W6^T + b
       (24 t-blocks x 8 k-tiles), written to DRAM scratch (bf16 planes).
    2. reads back Q^T/K^T/V^T as contiguous (64,3072) views (one 6KB
       descriptor per partition); V tiles via PE transposes (4 per DVE copy).
    3. per c-tile, the two heads' energy matmuls (K=64) go back-to-back into
       one PSUM tile at tile_position (0,0)/(64,0) -> the PE runs the pair
       concurrently in disjoint row groups (~1.75x).  exp(E) alternates
       between ScalarE (Exp activation, scale=128*ln2) and the Vector engine
       (Schraudolph bit-trick: ONE tensor_scalar computing
       int16(round(E'*2^14 + B)) whose bits are the bf16 approximation of
       exp(E), ~3% max rel err) so neither engine is the softmax bottleneck.
       One bf16 matmul per (c,head) with lhsT = [V_c | 1] (M=65) accumulates
       BOTH out^T[xi,r] and the softmax denominator Sigma[r] (row 64).
    4. writes per-head [out^T; Sigma] (65,3072) tiles per core.
  Host: divide rows 0:64 by row 64, interleave heads into (T,D), gamma*out+x.
  Toolchain workarounds: _split_multiwaits (this walrus allows one sync wait
  per instruction) and _install_ntff_shim (axon NTFF profiling hook).
"""

import sys

if "/opt/trn_rl_repo" not in sys.path:
    sys.path.insert(0, "/opt/trn_rl_repo")

import ml_dtypes
import numpy as np

BF16NP = ml_dtypes.bfloat16


def _install_ntff_shim():
    """concourse.bass_utils under axon imports antenv.axon_hooks when
    tracing is requested; this image's antenv lacks that submodule.
    Register an equivalent shim (backed by the boot image's ctypes NTFF
    driver) so BASS_TRACE=1 profiles instead of crashing."""
    import types

    if "antenv.axon_hooks" in sys.modules:
        return
    mod = types.ModuleType("antenv.axon_hooks")
    cell = {}

    def get_axon_ntff_profile_hook():
        if "h" not in cell:
            try:
                from trn_agent_boot.trn_boot import _ntff_profile_via_ctypes
                cell["h"] = _ntff_profile_via_ctypes("/opt/axon/libaxon_pjrt.so")
            except Exception:
                cell["h"] = None
        return cell["h"]

    def set_axon_ntff_profile_hook(h):
        cell["h"] = h

    mod.get_axon_ntff_profile_hook = get_axon_ntff_profile_hook
    mod.set_axon_ntff_profile_hook = set_axon_ntff_profile_hook
    sys.modules["antenv.axon_hooks"] = mod


_install_ntff_shim()

import concourse.bass as bass
import concourse.mybir as mybir
import concourse.tile as tile
from concourse.bass import ds, ts
from concourse.masks import make_identity

QSCALE = 1.4426950408889634 / 128.0   # log2e / 128 folded into Wq
ACT_EXP_SCALE = 128.0 * 0.6931471805599453  # recovers exp(E) on ScalarE

F32 = mybir.dt.float32
F32R = mybir.dt.float32r
BF16 = mybir.dt.bfloat16

T = 3072          # sequence length (and t2 size)
D = 1024          # model dim
H = 16            # heads
NCORE = 8
EG = 64           # channel groups per head (columns of Y_h)
XI = 64           # "feature" dim of the quirky attention (t // 48)
NKT = D // 128    # 8 contraction tiles for the projection
NTB = T // 128    # 24 t-blocks / c-tiles
RCH = 512         # r-chunk (free dim of energy/AV matmuls)
NR = T // RCH     # 6 r-chunks
W6 = 6 * EG       # 384 fused projection output columns
# c-tiles whose softmax exp runs on the Vector engine (Schraudolph bit-trick
# via one tensor_scalar) instead of ScalarE -- splits the softmax exp load.
DVE_SET = frozenset({1, 3, 5, 7, 9, 11, 13, 15, 17, 19, 21})
SCHRAUD_A = float(2.0 ** 14)          # 2^7 * 128 (E' is prescaled by 1/128)
SCHRAUD_B = 16250.368                 # (127 - 0.044) * 2^7


def _split_multiwaits(nc):
    """This toolchain's walrus accepts at most ONE sync wait per
    instruction (setupSyncWait: 'Too many sync wait commands'), but Tile
    attaches several. Hoist all but the last wait of each instruction onto
    same-engine NoOps inserted right before it — semantically identical
    (sem-ge waits executed in sequence)."""
    n = 0
    for fn in nc.m.functions:
        for bb in fn.blocks:
            insts = list(bb.instructions)
            out = []
            changed = False
            for inst in insts:
                si = inst.sync_info
                if si is not None and len(si.on_wait) > 1:
                    waits = list(si.on_wait)
                    for w in waits[:-1]:
                        n += 1
                        out.append(mybir.InstNoOp(
                            name=f"I-splitwait-{n}",
                            ins=[], outs=[], engine=inst.engine,
                            sync_info=mybir.SyncInfo(on_wait=[w], on_update=[]),
                        ))
                    inst.sync_info = mybir.SyncInfo(
                        on_wait=[waits[-1]], on_update=list(si.on_update)
                    )
                    changed = True
                out.append(inst)
            if changed:
                bb.instructions = out
    return n


def _coarsen_sem_incs(nc):
    """Tile attaches a sem-inc to EVERY instruction (its optimize_sems pass
    is disabled).  Each inc is a serializing EVT_SEM register write (~26ns)
    and breaks PE row-group matmul pairing.  Drop incs whose cumulative
    values no wait ever references, folding the dropped counts into the next
    kept inc -- sem values at every waited point are unchanged.  DMA sems
    (completion-fired) are left untouched."""
    import collections

    waitvals = collections.defaultdict(set)
    badsem = set()
    for fn in nc.m.functions:
        for bb in fn.blocks:
            for inst in bb.instructions:
                si = inst.sync_info
                if si is None:
                    continue
                for w in si.on_wait:
                    if (
                        w.sync_type != "semaphore"
                        or w.wait_mode != "sem-ge-imm"
                        or w.wait_reg is not None
                    ):
                        badsem.add(w.id)
                    else:
                        waitvals[w.id].add(w.wait_value)

    upd_sites = collections.defaultdict(list)
    for fn in nc.m.functions:
        for bb in fn.blocks:
            for inst in bb.instructions:
                si = inst.sync_info
                if si is None:
                    continue
                for u in si.on_update:
                    if (
                        u.sync_type != "semaphore"
                        or u.update_mode != "sem-inc"
                        or u.update_reg is not None
                        or (u.ant_name or "").startswith("DMA")
                    ):
                        badsem.add(u.id)
                        continue
                    upd_sites[u.id].append((id(bb), inst, u))

    for sid, sites in upd_sites.items():
        if len({b for b, _, _ in sites}) > 1:
            badsem.add(sid)
        if len({inst.engine for _, inst, _ in sites}) > 1:
            badsem.add(sid)

    # walrus requires every sem-inc to have update_value == 1, so dropped
    # incs cannot be folded into a bigger one.  Instead drop them outright
    # and renumber every wait on that sem to the new (smaller) counts.
    ndrop = 0
    dropset = set()          # ids of SyncUpdate objects to drop
    remap = {}               # sem id -> sorted list of kept cum values
    for sid, sites in upd_sites.items():
        if sid in badsem:
            continue
        wv = waitvals.get(sid, set())
        cum = 0
        kept_cums = []
        for i, (_, inst, u) in enumerate(sites):
            cum += u.update_value
            if (cum in wv) or (i == len(sites) - 1):
                kept_cums.append(cum)
            else:
                dropset.add(id(u))
                ndrop += 1
        remap[sid] = kept_cums

    import bisect

    for fn in nc.m.functions:
        for bb in fn.blocks:
            for inst in bb.instructions:
                si = inst.sync_info
                if si is None:
                    continue
                new_upd = [u for u in si.on_update if id(u) not in dropset]
                new_wait = []
                for w in si.on_wait:
                    if w.id in remap:
                        kept = remap[w.id]
                        # new value = rank of first kept cum >= old value
                        nv = bisect.bisect_left(kept, w.wait_value) + 1
                        nv = min(nv, len(kept))
                        if nv != w.wait_value:
                            w = mybir.SyncWait(
                                sync_type=w.sync_type,
                                id=w.id,
                                ant_name=w.ant_name,
                                wait_mode=w.wait_mode,
                                wait_value=nv,
                                wait_reg=w.wait_reg,
                            )
                    new_wait.append(w)
                if len(new_upd) != len(si.on_update) or any(
                    a is not b for a, b in zip(new_wait, si.on_wait)
                ):
                    inst.sync_info = mybir.SyncInfo(
                        on_wait=new_wait, on_update=new_upd
                    )
    return ndrop


def build_program():
    nc = bass.Bass()

    xT = nc.dram_tensor("xT", [NTB, 128, NKT, 128], BF16, kind="ExternalInput")
    w6 = nc.dram_tensor("w6", [D, W6], BF16, kind="ExternalInput")
    b6 = nc.dram_tensor("b6", [128, W6], F32, kind="ExternalInput")
    y6qk = nc.dram_tensor("y6qk", [4, T, EG], BF16, kind="Internal")
    y6v = nc.dram_tensor("y6v", [2, T, EG], BF16, kind="Internal")
    outT = nc.dram_tensor("outT", [2, XI + 1, T], F32, kind="ExternalOutput")

    with tile.TileContext(nc) as tc:
        with tc.tile_pool(name="const", bufs=1) as constp:
            w6_sb = constp.tile([128, NKT, W6], BF16)
            w6v = w6[:, :].rearrange("(k p) n -> k p n", p=128)
            for k in range(NKT):
                nc.scalar.dma_start(out=w6_sb[:, k, :], in_=w6v[k, :, :])
            b6_sb = constp.tile([128, W6], F32)
            nc.scalar.dma_start(out=b6_sb, in_=b6[:, :])
            # identity blocks at partitions 0:64 and 64:128 so the two heads'
            # V^T transposes run row-paired in the PE array
            ident = constp.tile([128, 64], BF16)
            nc.gpsimd.memset(ident, 0.0)
            make_identity(nc, ident[0:64, :], nomemset=True)
            make_identity(nc, ident[64:128, :], nomemset=True)
            ones_f32 = constp.tile([128, 1], F32)
            nc.gpsimd.memset(ones_f32, 1.0)
            kt_sb = constp.tile([128, T], BF16)   # rows 0:64 h1 K^T, 64:128 h2
            qt_sb = constp.tile([128, T], BF16)   # rows 0:64 h1 Q^T, 64:128 h2
            vt_sb = constp.tile([128, T], BF16)   # rows 0:64 h1 V^T, 64:128 h2
            # V tiles augmented with a ones column: [:, c, 0:64] = V_h c-tile,
            # [:, c, 64] = 1.0 so one matmul computes out^T AND Sigma (row 64)
            v1_sb = constp.tile([128, NTB, XI + 1], BF16)
            v2_sb = constp.tile([128, NTB, XI + 1], BF16)
            for vsb in (v1_sb, v2_sb):
                for c in range(NTB):
                    nc.vector.tensor_copy(vsb[:, c, XI:XI + 1], ones_f32)

            # ---------------- projection: Y6 = X @ W6^T + b6 ----------------
            with tc.tile_pool(name="xt", bufs=6) as xtp, \
                 tc.tile_pool(name="psy", bufs=4, space="PSUM") as psyp, \
                 tc.tile_pool(name="ysb", bufs=4) as ysbp:
                for j in range(NTB):
                    xt = xtp.tile([128, NKT, 128], BF16)
                    (nc.sync if j % 2 == 0 else nc.scalar).dma_start(
                        out=xt.rearrange("p k t -> p (k t)"),
                        in_=xT[j, :, :, :].rearrange("p k t -> p (k t)"),
                    )
                    psy = psyp.tile([128, W6], F32)
                    for k in range(NKT):
                        nc.tensor.matmul(
                            psy, xt[:, k, :], w6_sb[:, k, :],
                            start=(k == 0), stop=(k == NKT - 1),
                        )
                    psyv = psy.rearrange("p (h z e) -> p h z e", h=2, z=3)
                    b6v = b6_sb.rearrange("p (h z e) -> p h z e", h=2, z=3)
                    ysbqk = ysbp.tile([128, 2, 2, EG], BF16, name="ysbqk")
                    nc.vector.tensor_add(ysbqk, psyv[:, :, 0:2, :],
                                         b6v[:, :, 0:2, :])
                    ysbv = ysbp.tile([128, 2, EG], BF16, name="ysbv")
                    nc.vector.tensor_add(ysbv, psyv[:, :, 2, :],
                                         b6v[:, :, 2, :])
                    nc.scalar.dma_start(
                        out=y6qk[:, ts(j, 128), :].rearrange("q t e -> t q e"),
                        in_=ysbqk,
                    )
                    nc.sync.dma_start(
                        out=y6v[:, ts(j, 128), :].rearrange("q t e -> t q e"),
                        in_=ysbv,
                    )

            # ------- load Q^T / K^T / V^T as contiguous (64, 3072) views.
            # Flat 2D APs -> one 6KB descriptor per partition (NOT 48x128B).
            engs = (nc.sync, nc.scalar)
            ldn = 0
            for srcap, bufap in (
                    (y6v[0, :, :], vt_sb[0:64, :]),
                    (y6v[1, :, :], vt_sb[64:128, :]),
                    (y6qk[1, :, :], kt_sb[0:64, :]),
                    (y6qk[3, :, :], kt_sb[64:128, :]),
                    (y6qk[0, :, :], qt_sb[0:64, :]),
                    (y6qk[2, :, :], qt_sb[64:128, :])):
                engs[ldn % 2].dma_start(
                    out=bufap,
                    in_=srcap.rearrange("(xi a) e -> xi (a e)", xi=64),
                )
                ldn += 1

            # ------- V tiles: true transpose of V^T chunks via the PE -------
            # 4 c-tiles batched per PSUM tile so one DVE copy moves 4 blocks
            with tc.tile_pool(name="vtps", bufs=4, space="PSUM") as vtpsp:
                for c0 in range(0, NTB, 4):
                    for vsb, row0 in ((v1_sb, 0), (v2_sb, 64)):
                        vp = vtpsp.tile([128, 4, XI], BF16)
                        for i in range(4):
                            nc.tensor.transpose(
                                vp[:, i, :],
                                vt_sb[row0:row0 + 64, ts(c0 + i, 128)],
                                ident[row0:row0 + 64, :],
                            )
                        nc.vector.tensor_copy(
                            vsb[:, ds(c0, 4), 0:XI], vp
                        )

            # --------------------------- attention --------------------------
            # Per c-tile the two heads' energy matmuls are emitted back to
            # back into the same PSUM tile (disjoint PE row groups 0:64 /
            # 64:128 -> the array runs them concurrently).  Each (128, 1024)
            # energy pair is exponentiated on ScalarE (exp) OR on the Vector
            # engine (custom poly^128 ops) per DVE_SET to split the softmax
            # exp across both engines.
            with tc.tile_pool(name="eps", bufs=3, space="PSUM") as epp, \
                 tc.tile_pool(name="ex", bufs=8) as expool, \
                 tc.tile_pool(name="outp", bufs=1, space="PSUM") as outpp, \
                 tc.tile_pool(name="osb", bufs=4) as osbp:
                # One flat software-pipelined stream over all (r, c) tiles:
                # the AV consumption lags the energy/exp production by LAG
                # tiles ACROSS r boundaries, so the exp engines always lead
                # and the energy-PSUM ring never starves at a chunk edge.
                LAG = 3
                NT = NR * NTB
                exq = {}
                outs = {}
                for cc in range(NT + LAG):
                    if cc < NT:
                        r_p, c_p = divmod(cc, NTB)
                        ep = epp.tile([128, 2, RCH], F32)
                        nc.tensor.matmul(
                            ep[:, 0, :], kt_sb[0:64, ts(c_p, 128)],
                            qt_sb[0:64, ts(r_p, RCH)], start=True, stop=True,
                        )
                        nc.tensor.matmul(
                            ep[:, 1, :], kt_sb[64:128, ts(c_p, 128)],
                            qt_sb[64:128, ts(r_p, RCH)], start=True, stop=True,
                        )
                        ex = expool.tile([128, 2, RCH], BF16)
                        if c_p in DVE_SET:
                            nc.vector.tensor_scalar(
                                ex.bitcast(mybir.dt.int16), ep,
                                SCHRAUD_A, SCHRAUD_B,
                                mybir.AluOpType.mult, mybir.AluOpType.add,
                            )
                        else:
                            nc.scalar.activation(
                                ex, ep, mybir.ActivationFunctionType.Exp,
                                scale=ACT_EXP_SCALE,
                            )
                        exq[cc] = ex
                    if cc >= LAG:
                        r_c, c_c = divmod(cc - LAG, NTB)
                        if c_c == 0:
                            outs[r_c] = (
                                outpp.tile([XI + 1, RCH], F32, name="o1"),
                                outpp.tile([XI + 1, RCH], F32, name="o2"),
                            )
                        outp1, outp2 = outs[r_c]
                        ex = exq.pop(cc - LAG)
                        nc.tensor.matmul(
                            outp1, v1_sb[:, c_c, :], ex[:, 0, :],
                            start=(c_c == 0), stop=(c_c == NTB - 1),
                        )
                        nc.tensor.matmul(
                            outp2, v2_sb[:, c_c, :], ex[:, 1, :],
                            start=(c_c == 0), stop=(c_c == NTB - 1),
                        )
                        if c_c == NTB - 1:
                            for outp, hl in ((outp1, 0), (outp2, 1)):
                                osb = osbp.tile([XI + 1, RCH], F32)
                                nc.vector.tensor_copy(
                                    osb[:, 0:RCH // 2], outp[:, 0:RCH // 2]
                                )
                                nc.scalar.copy(
                                    osb[:, RCH // 2:RCH], outp[:, RCH // 2:RCH]
                                )
                                nc.gpsimd.dma_start(
                                    out=outT[hl, :, ts(r_c, RCH)], in_=osb
                                )
                            del outs[r_c]
    return nc


def make_in_maps(x, Wq, bq, Wk, bk, Wv, bv):
    X = np.ascontiguousarray(np.asarray(x, dtype=np.float32).reshape(T, D))
    # (NTB, 128, NKT, 128): [j, p, k, t] = X[128j+t, 128k+p] -- every SBUF
    # partition reads one contiguous 2KB run per projection slab DMA
    xTm = np.ascontiguousarray(
        X.astype(BF16NP).reshape(NTB, 128, NKT, 128).transpose(0, 3, 2, 1)
    )
    in_maps = []
    for c in range(NCORE):
        wcols, bcols = [], []
        for h in (2 * c, 2 * c + 1):
            for W, b, s in ((Wq, bq, QSCALE), (Wk, bk, 1.0), (Wv, bv, 1.0)):
                wcols.append(np.asarray(W, np.float32)[h::H, :].T * np.float32(s))
                bcols.append(np.asarray(b, np.float32)[h::H] * np.float32(s))
        w6m = np.ascontiguousarray(np.concatenate(wcols, axis=1).astype(BF16NP))
        b6m = np.ascontiguousarray(
            np.broadcast_to(np.concatenate(bcols), (128, W6)).astype(np.float32)
        )
        in_maps.append({"xT": xTm, "w6": w6m, "b6": b6m})
    return X, in_maps


def assemble(X, results, gamma):
    O = np.empty((T, EG, H), dtype=np.float32)
    for c in range(NCORE):
        res = results[c]
        for hl in range(2):
            h = 2 * c + hl
            onn = res["outT"][hl][0:XI, :]                # (64, 3072)
            s = res["outT"][hl][XI, :]                    # (3072,)
            O[:, :, h] = (onn / s[None, :]).T
    out = O.reshape(T, D)
    g = np.float32(np.asarray(gamma))
    return (g * out + X).reshape(1, 1, T, D).astype(np.float32)


_PROGRAM = None
last_run_info = {}


def kernel(x, Wq, bq, Wk, bk, Wv, bv, gamma):
    global _PROGRAM
    from concourse import bass_utils

    X, in_maps = make_in_maps(x, Wq, bq, Wk, bk, Wv, bv)
    if _PROGRAM is None:
        import os

        _PROGRAM = build_program()
        if os.environ.get("COARSEN") == "1":
            _coarsen_sem_incs(_PROGRAM)
        # required for this toolchain's walrus (1 sync wait per instruction);
        # applied here so CoreSim (which predates these NoOps) can still run
        # the unsplit program from build_program()
        _split_multiwaits(_PROGRAM)
    res = bass_utils.run_bass_kernel_spmd(
        _PROGRAM, in_maps, core_ids=list(range(NCORE))
    )
    last_run_info["exec_time_ns"] = res.exec_time_ns
    last_run_info["trace"] = res.instructions_and_trace
    return assemble(X, res.results, gamma)

